# revision 2
# baseline (speedup 1.0000x reference)
"""Trainium2 Bass kernel for nn_GATNetMultiLayer (2x GAT + dense stack + cdist).

Sharding: dst-nodes row-partitioned across 8 cores (N/8 rows each); edges
bucketed per 128-dst window on host and padded to a uniform tile count so the
SPMD program is fully static. GAT segment-softmax aggregation runs as one-hot
matmuls on the tensor engine (lhsT = per-edge one-hot * exp(score), rhs =
dma_gather'ed source features); the softmax denominator comes from eq.T @ ex
matmuls into a second PSUM bank. Layer-2 features + attention scores are
exchanged with one AllGather; final [N,3] coordinates with a tiny AllGather;
each core then writes its [N/8, N] row block of the distance matrix.
"""
import sys
sys.path.insert(0, "/opt/trn_rl_repo")
import numpy as np

import concourse.bass as bass
import concourse.bacc as bacc
import concourse.mybir as mybir
import concourse.tile as tile
from concourse import bass_utils

dt = mybir.dt
Alu = mybir.AluOpType
Act = mybir.ActivationFunctionType

NCORES = 8
LN_EPS = 1e-5


class Cfg:
    def __init__(self, n, t_win, gch=6, f32_cdist_mm=True, nt_limit=None):
        self.N = n
        self.RPC = n // NCORES
        self.NW = self.RPC // 128
        self.T = t_win
        self.EW = t_win * 128
        self.F0 = 512
        self.F1 = 512
        self.F2 = 256
        self.GCH = min(gch, t_win)
        self.f32_cdist_mm = f32_cdist_mm
        self.nt_limit = nt_limit
        self.fw_limit = None


# ----------------------------------------------------------------------------
def build_kernel(c: Cfg, phases="ABCDE"):
    nc = bacc.Bacc("TRN2", target_bir_lowering=False, debug=False,
                   num_devices=NCORES)

    def inp(name, shape, dtype=dt.float32):
        return nc.dram_tensor(name, shape, dtype, kind="ExternalInput")

    xT = inp("xT", [c.F0, c.N], dt.float32r)
    gidx = inp("gidx", [c.NW, 128, c.EW // 16], dt.int16)
    didx = inp("didx", [c.NW, 128, c.EW // 16], dt.int16)
    dstrel = inp("dstrel", [c.NW, 128, c.T])
    W1c = inp("W1c", [4, 128, c.F1], dt.float32r)
    Vs1 = inp("Vs1", [4, 128, 4], dt.float32r)
    W2c = inp("W2c", [4, 128, c.F2], dt.float32r)
    Vs2 = inp("Vs2", [4, 128, 4], dt.float32r)
    Wal2 = inp("Wal2", [4, 128, 256], dt.float32r)
    Wda = inp("Wda", [2, 128, 128], dt.float32r)
    Wada = inp("Wada", [2, 128, 128], dt.float32r)
    Wd1 = inp("Wd1", [128, 64], dt.float32r)
    Wad1 = inp("Wad1", [128, 64], dt.float32r)
    Wd2 = inp("Wd2", [64, 32], dt.float32r)
    Wd3 = inp("Wd3", [32, 4], dt.float32r)
    Bb1 = inp("Bb1", [128, c.F1])
    Bb2 = inp("Bb2", [128, c.F2])
    Bbal2 = inp("Bbal2", [128, 256])
    Bbda = inp("Bbda", [128, 128])
    Bbada = inp("Bbada", [128, 128])
    Bga = inp("Bga", [128, 128])
    Bbea = inp("Bbea", [128, 128])
    Bbd1 = inp("Bbd1", [128, 64])
    Bbad1 = inp("Bbad1", [128, 64])
    Bg1 = inp("Bg1", [128, 64])
    Bbe1 = inp("Bbe1", [128, 64])
    Bbd2 = inp("Bbd2", [128, 32])
    Bg2 = inp("Bg2", [128, 32])
    Bbe2 = inp("Bbe2", [128, 32])
    Bbd3 = inp("Bbd3", [128, 3])

    out = nc.dram_tensor("out", [c.RPC, c.N], dt.float32, kind="ExternalOutput")

    cd_dt = dt.float32 if c.f32_cdist_mm else dt.float32r
    h1_pre = nc.dram_tensor("h1_pre", [c.N, c.F1], dt.float32r)
    sc1_pad = nc.dram_tensor("sc1_pad", [c.N, 64], dt.float32)
    h2_ag_in = nc.dram_tensor("h2_ag_in", [c.RPC, c.F2], dt.float32r)
    h2_full = nc.dram_tensor("h2_full", [c.N, c.F2], dt.float32r,
                             addr_space="Shared")
    sc2_ag_in = nc.dram_tensor("sc2_ag_in", [c.RPC, 64], dt.float32)
    sc2_full = nc.dram_tensor("sc2_full", [c.N, 64], dt.float32,
                              addr_space="Shared")
    cdT_out = nc.dram_tensor("cdT_out", [4, c.RPC], dt.float32,
                             kind="ExternalOutput")
    sq_out = nc.dram_tensor("sq_out", [128, c.NW], dt.float32,
                            kind="ExternalOutput")

    NT = c.N // 128
    rg = [list(range(NCORES))]

    with tile.TileContext(nc) as tc:
      with tc.tile_pool(name="const", bufs=1) as cpool:
        iota_f = cpool.tile([128, 128], dt.float32)
        nc.gpsimd.iota(iota_f[:], pattern=[[1, 128]], base=0,
                       channel_multiplier=0,
                       allow_small_or_imprecise_dtypes=True)
        ident = cpool.tile([128, 128], dt.float32)
        iota_p = cpool.tile([128, 1], dt.float32)
        nc.gpsimd.iota(iota_p[:], pattern=[[0, 1]], base=0,
                       channel_multiplier=1,
                       allow_small_or_imprecise_dtypes=True)
        nc.vector.tensor_scalar(ident[:], iota_f[:], iota_p[:], None,
                                op0=Alu.is_equal)

        h_own = cpool.tile([128, c.NW, c.F1], dt.float32)
        xi_own = cpool.tile([128, c.NW, 256], dt.float32)
        h2_own = cpool.tile([128, c.NW, c.F2], dt.float32)
        sq_own = cpool.tile([128, c.NW], dt.float32)

        # ================= phase A: h1_pre + scores1 for all nodes =========
        if "A" in phases:
         with tc.tile_pool(name="pac", bufs=1) as pac, \
             tc.tile_pool(name="pha", bufs=3) as pa, \
             tc.tile_pool(name="pha_ps", bufs=2, space="PSUM") as pap:
            w1_sb = pac.tile([128, 4, c.F1], dt.float32r)
            nc.sync.dma_start(w1_sb[:], W1c.ap().rearrange("c p f -> p c f"))
            vs1_sb = pac.tile([128, 4, 4], dt.float32r)
            nc.sync.dma_start(vs1_sb[:], Vs1.ap().rearrange("c p f -> p c f"))
            for ntile in range(min(NT, c.nt_limit or NT)):
                xc = pa.tile([128, 4, 128], dt.float32r, tag="xc")
                nc.sync.dma_start(
                    xc[:],
                    xT.ap().rearrange("(k p) n -> p k n", p=128)[
                        :, :, ntile * 128:(ntile + 1) * 128])
                ph = pap.tile([128, c.F1], dt.float32, tag="ph")
                psc = pap.tile([128, 4], dt.float32, tag="psc")
                for k in range(4):
                    nc.tensor.matmul(ph[:], xc[:, k, :], w1_sb[:, k, :],
                                     start=(k == 0), stop=(k == 3))
                for k in range(4):
                    nc.tensor.matmul(psc[:], xc[:, k, :], vs1_sb[:, k, :],
                                     start=(k == 0), stop=(k == 3))
                h_sb = pa.tile([128, c.F1], dt.float32r, tag="h_sb")
                nc.vector.tensor_copy(h_sb[:], ph[:])
                sc_sb = pa.tile([128, 64], dt.float32, tag="sc_sb")
                nc.vector.tensor_copy(sc_sb[:, 0:4], psc[:])
                nc.vector.memset(sc_sb[:, 4:64], 0.0)
                nc.sync.dma_start(
                    h1_pre.ap()[ntile * 128:(ntile + 1) * 128, :], h_sb[:])
                nc.sync.dma_start(
                    sc1_pad.ap()[ntile * 128:(ntile + 1) * 128, :], sc_sb[:])

        # ================= GAT window helper ===============================
        def gat_window(w, src_tab, sc_tab, F, b_sb, out_ap, pg, pgp):
            D = F // 2
            drel = pg.tile([128, c.T], dt.float32, tag="drel")
            nc.sync.dma_start(drel[:], dstrel.ap()[w, :, :])
            gi = pg.tile([128, c.EW // 16], dt.int16, tag="gi")
            nc.sync.dma_start(gi[:], gidx.ap()[w, :, :])
            di = pg.tile([128, c.EW // 16], dt.int16, tag="di")
            nc.sync.dma_start(di[:], didx.ap()[w, :, :])
            sa = pg.tile([128, c.T, 64], dt.float32, tag="sa")
            sd = pg.tile([128, c.T, 64], dt.float32, tag="sd")
            for s0 in range(0, c.T, c.GCH):
                sn = min(c.GCH, c.T - s0)
                nc.gpsimd.dma_gather(sa[:, s0:s0 + sn, :], sc_tab.ap(),
                                     gi[:, s0 * 8:(s0 + sn) * 8],
                                     num_idxs=sn * 128, num_idxs_reg=sn * 128,
                                     elem_size=64)
                nc.gpsimd.dma_gather(sd[:, s0:s0 + sn, :], sc_tab.ap(),
                                     di[:, s0 * 8:(s0 + sn) * 8],
                                     num_idxs=sn * 128, num_idxs_reg=sn * 128,
                                     elem_size=64)
            ex = pg.tile([128, c.T, 2], dt.float32, tag="ex")
            nc.vector.tensor_tensor(ex[:], sa[:, :, 0:2], sd[:, :, 2:4],
                                    op=Alu.add)
            nc.vector.scalar_tensor_tensor(ex[:], ex[:], 0.2, ex[:],
                                           op0=Alu.mult, op1=Alu.max)
            nc.scalar.activation(ex[:], ex[:], Act.Exp)
            ex_r = pg.tile([128, c.T, 2], dt.float32r, tag="ex_r")
            nc.vector.tensor_copy(ex_r[:], ex[:])

            pagg0 = pgp.tile([128, D], dt.float32, tag="pagg0")
            pagg1 = pgp.tile([128, D], dt.float32, tag="pagg1")
            pagg = [pagg0, pagg1]
            ps = pgp.tile([128, 2], dt.float32, tag="ps")
            nchunk = (c.T + c.GCH - 1) // c.GCH
            for ch in range(nchunk):
                t0 = ch * c.GCH
                tn = min(c.GCH, c.T - t0)
                G = pg.tile([128, c.GCH, F], dt.float32r, tag="G")
                nc.gpsimd.dma_gather(
                    G[:, 0:tn, :], src_tab.ap(), gi[:, t0 * 8:(t0 + tn) * 8],
                    num_idxs=tn * 128, num_idxs_reg=tn * 128, elem_size=F)
                for tt in range(tn):
                    t = t0 + tt
                    eq = pg.tile([128, 128], dt.float32r, tag="eq")
                    nc.vector.tensor_scalar(eq[:], iota_f[:],
                                            drel[:, t:t + 1], None,
                                            op0=Alu.is_equal)
                    nc.tensor.matmul(ps[:], eq[:], ex_r[:, t, :],
                                     start=(t == 0), stop=(t == c.T - 1))
                    for h in range(2):
                        sal = pg.tile([128, 128], dt.float32r, tag=f"sal{h}")
                        nc.vector.tensor_scalar(sal[:], eq[:],
                                                ex[:, t, h:h + 1], None,
                                                op0=Alu.mult)
                        nc.tensor.matmul(pagg[h][:], sal[:],
                                         G[:, tt, h * D:(h + 1) * D],
                                         start=(t == 0), stop=(t == c.T - 1))
            rcp = pg.tile([128, 2], dt.float32, tag="rcp")
            nc.vector.reciprocal(rcp[:], ps[:])
            for h in range(2):
                nc.vector.scalar_tensor_tensor(
                    out_ap[:, h * D:(h + 1) * D], pagg[h][:],
                    rcp[:, h:h + 1], b_sb[:, h * D:(h + 1) * D],
                    op0=Alu.mult, op1=Alu.add)

        # ================= phase B: GAT1 ==================================
        if "B" in phases:
         with tc.tile_pool(name="pbc", bufs=1) as pbc, \
             tc.tile_pool(name="phb", bufs=2) as pb, \
             tc.tile_pool(name="phb_ps", bufs=2, space="PSUM") as pbp:
            b1_sb = pbc.tile([128, c.F1], dt.float32)
            nc.sync.dma_start(b1_sb[:], Bb1.ap())
            for w in range(c.NW):
                gat_window(w, h1_pre, sc1_pad, c.F1, b1_sb,
                           h_own[:, w, :], pb, pbp)
                nc.vector.tensor_scalar(h_own[:, w, :], h_own[:, w, :],
                                        0.0, None, op0=Alu.max)

        # ================= phase C: h2_pre/xi/scores2 + AllGather =========
        if "C" in phases:
         with tc.tile_pool(name="pcc", bufs=1) as pcc, \
             tc.tile_pool(name="phc", bufs=3) as pc, \
             tc.tile_pool(name="phc_ps", bufs=2, space="PSUM") as pcp:
            w2_sb = pcc.tile([128, 4, c.F2], dt.float32r)
            nc.sync.dma_start(w2_sb[:], W2c.ap().rearrange("c p f -> p c f"))
            vs2_sb = pcc.tile([128, 4, 4], dt.float32r)
            nc.sync.dma_start(vs2_sb[:], Vs2.ap().rearrange("c p f -> p c f"))
            wal2_sb = pcc.tile([128, 4, 256], dt.float32r)
            nc.sync.dma_start(wal2_sb[:], Wal2.ap().rearrange("c p f -> p c f"))
            bal2_sb = pcc.tile([128, 256], dt.float32)
            nc.sync.dma_start(bal2_sb[:], Bbal2.ap())
            hT = pcc.tile([128, 4, c.RPC], dt.float32r)
            for w in range(c.NW):
                for k in range(4):
                    pt = pcp.tile([128, 128], dt.float32, tag="pt")
                    nc.tensor.transpose(pt[:],
                                        h_own[:, w, k * 128:(k + 1) * 128],
                                        ident[:])
                    nc.vector.tensor_copy(hT[:, k, w * 128:(w + 1) * 128],
                                          pt[:])
            for w in range(c.NW):
                ph2 = pcp.tile([128, c.F2], dt.float32, tag="ph2")
                pxi = pcp.tile([128, 256], dt.float32, tag="pxi")
                psc2 = pcp.tile([128, 4], dt.float32, tag="psc2")
                for k in range(4):
                    lhs = hT[:, k, w * 128:(w + 1) * 128]
                    nc.tensor.matmul(ph2[:], lhs, w2_sb[:, k, :],
                                     start=(k == 0), stop=(k == 3))
                    nc.tensor.matmul(pxi[:], lhs, wal2_sb[:, k, :],
                                     start=(k == 0), stop=(k == 3))
                    nc.tensor.matmul(psc2[:], lhs, vs2_sb[:, k, :],
                                     start=(k == 0), stop=(k == 3))
                h2s = pc.tile([128, c.F2], dt.float32r, tag="h2s")
                nc.vector.tensor_copy(h2s[:], ph2[:])
                nc.sync.dma_start(h2_ag_in.ap()[w * 128:(w + 1) * 128, :],
                                  h2s[:])
                nc.vector.tensor_tensor(xi_own[:, w, :], pxi[:], bal2_sb[:],
                                        op=Alu.add)
                sc2s = pc.tile([128, 64], dt.float32, tag="sc2s")
                nc.vector.tensor_copy(sc2s[:, 0:4], psc2[:])
                nc.vector.memset(sc2s[:, 4:64], 0.0)
                nc.sync.dma_start(sc2_ag_in.ap()[w * 128:(w + 1) * 128, :],
                                  sc2s[:])
            nc.gpsimd.collective_compute(
                "AllGather", Alu.bypass, replica_groups=rg,
                ins=[h2_ag_in.ap().opt()], outs=[h2_full.ap().opt()])
            nc.gpsimd.collective_compute(
                "AllGather", Alu.bypass, replica_groups=rg,
                ins=[sc2_ag_in.ap().opt()], outs=[sc2_full.ap().opt()])

        # ================= phase D: GAT2 ==================================
        if "D" in phases:
         with tc.tile_pool(name="pdc", bufs=1) as pdc, \
             tc.tile_pool(name="phd", bufs=2) as pd, \
             tc.tile_pool(name="phd_ps", bufs=2, space="PSUM") as pdp:
            b2_sb = pdc.tile([128, c.F2], dt.float32)
            nc.sync.dma_start(b2_sb[:], Bb2.ap())
            for w in range(c.NW):
                gat_window(w, h2_full, sc2_full, c.F2, b2_sb,
                           h2_own[:, w, :], pd, pdp)
                nc.vector.scalar_tensor_tensor(
                    h2_own[:, w, :], h2_own[:, w, :], 0.0, xi_own[:, w, :],
                    op0=Alu.max, op1=Alu.add)

        # ================= phase E: dense stack ===========================
        def ln_relu_resid(out_ap, pin, g_sb, be_sb, b_sb, xi_ap, D, pe):
            x = pe.tile([128, D], dt.float32, tag=f"lnx{D}")
            nc.vector.tensor_tensor(x[:], pin[:], b_sb[:, 0:D], op=Alu.add)
            s1 = pe.tile([128, 1], dt.float32, tag=f"ls{D}")
            nc.vector.reduce_sum(s1[:], x[:], axis=mybir.AxisListType.X)
            mu = pe.tile([128, 1], dt.float32, tag=f"lm{D}")
            nc.vector.tensor_scalar(mu[:], s1[:], 1.0 / D, None, op0=Alu.mult)
            sq = pe.tile([128, D], dt.float32, tag=f"lq{D}")
            ss = pe.tile([128, 1], dt.float32, tag=f"lss{D}")
            nc.scalar.activation(sq[:], x[:], Act.Square, accum_out=ss[:])
            v1 = pe.tile([128, 1], dt.float32, tag=f"lv{D}")
            nc.vector.tensor_scalar(v1[:], ss[:], 1.0 / D, None, op0=Alu.mult)
            mu2 = pe.tile([128, 1], dt.float32, tag=f"lm2{D}")
            nc.vector.tensor_scalar(mu2[:], mu[:], mu[:], None, op0=Alu.mult)
            var = pe.tile([128, 1], dt.float32, tag=f"lvr{D}")
            nc.vector.tensor_tensor(var[:], v1[:], mu2[:], op=Alu.subtract)
            nc.vector.tensor_scalar(var[:], var[:], LN_EPS, None, op0=Alu.add)
            std = pe.tile([128, 1], dt.float32, tag=f"lsd{D}")
            nc.scalar.activation(std[:], var[:], Act.Sqrt)
            rstd = pe.tile([128, 1], dt.float32, tag=f"lrs{D}")
            nc.vector.reciprocal(rstd[:], std[:])
            nrm = pe.tile([128, D], dt.float32, tag=f"lnr{D}")
            nc.vector.tensor_scalar(nrm[:], x[:], mu[:], rstd[:],
                                    op0=Alu.subtract, op1=Alu.mult)
            nc.vector.tensor_tensor(nrm[:], nrm[:], g_sb[:, 0:D], op=Alu.mult)
            nc.vector.tensor_tensor(nrm[:], nrm[:], be_sb[:, 0:D], op=Alu.add)
            if xi_ap is not None:
                nc.vector.scalar_tensor_tensor(out_ap, nrm[:], 0.0, xi_ap,
                                               op0=Alu.max, op1=Alu.add)
            else:
                nc.vector.tensor_scalar(out_ap, nrm[:], 0.0, None, op0=Alu.max)

        if "E" in phases:
         with tc.tile_pool(name="pec", bufs=1) as pec, \
             tc.tile_pool(name="phe", bufs=3) as pe, \
             tc.tile_pool(name="phe_ps", bufs=2, space="PSUM") as pep:
            wda_sb = pec.tile([128, 2, 128], dt.float32r)
            nc.sync.dma_start(wda_sb[:], Wda.ap().rearrange("c p f -> p c f"))
            wada_sb = pec.tile([128, 2, 128], dt.float32r)
            nc.sync.dma_start(wada_sb[:], Wada.ap().rearrange("c p f -> p c f"))
            wd1_sb = pec.tile([128, 64], dt.float32r)
            nc.sync.dma_start(wd1_sb[:], Wd1.ap())
            wad1_sb = pec.tile([128, 64], dt.float32r)
            nc.sync.dma_start(wad1_sb[:], Wad1.ap())
            wd2_sb = pec.tile([64, 32], dt.float32r)
            nc.sync.dma_start(wd2_sb[:], Wd2.ap())
            wd3_sb = pec.tile([32, 4], dt.float32r)
            nc.sync.dma_start(wd3_sb[:], Wd3.ap())
            bda_sb = pec.tile([128, 128], dt.float32)
            nc.sync.dma_start(bda_sb[:], Bbda.ap())
            bada_sb = pec.tile([128, 128], dt.float32)
            nc.sync.dma_start(bada_sb[:], Bbada.ap())
            ga_sb = pec.tile([128, 128], dt.float32)
            nc.sync.dma_start(ga_sb[:], Bga.ap())
            bea_sb = pec.tile([128, 128], dt.float32)
            nc.sync.dma_start(bea_sb[:], Bbea.ap())
            bd1_sb = pec.tile([128, 64], dt.float32)
            nc.sync.dma_start(bd1_sb[:], Bbd1.ap())
            bad1_sb = pec.tile([128, 64], dt.float32)
            nc.sync.dma_start(bad1_sb[:], Bbad1.ap())
            g1_sb = pec.tile([128, 64], dt.float32)
            nc.sync.dma_start(g1_sb[:], Bg1.ap())
            be1_sb = pec.tile([128, 64], dt.float32)
            nc.sync.dma_start(be1_sb[:], Bbe1.ap())
            bd2_sb = pec.tile([128, 32], dt.float32)
            nc.sync.dma_start(bd2_sb[:], Bbd2.ap())
            g2_sb = pec.tile([128, 32], dt.float32)
            nc.sync.dma_start(g2_sb[:], Bg2.ap())
            be2_sb = pec.tile([128, 32], dt.float32)
            nc.sync.dma_start(be2_sb[:], Bbe2.ap())
            bd3_sb = pec.tile([128, 3], dt.float32)
            nc.sync.dma_start(bd3_sb[:], Bbd3.ap())

            h2T = pec.tile([128, 2, c.RPC], dt.float32r)
            for w in range(c.NW):
                for k in range(2):
                    pt = pep.tile([128, 128], dt.float32, tag="ptE")
                    nc.tensor.transpose(pt[:],
                                        h2_own[:, w, k * 128:(k + 1) * 128],
                                        ident[:])
                    nc.vector.tensor_copy(h2T[:, k, w * 128:(w + 1) * 128],
                                          pt[:])
            t_a = pec.tile([128, c.NW, 128], dt.float32)
            for w in range(c.NW):
                pda = pep.tile([128, 128], dt.float32, tag="pmA")
                pada = pep.tile([128, 128], dt.float32, tag="pmB")
                for k in range(2):
                    lhs = h2T[:, k, w * 128:(w + 1) * 128]
                    nc.tensor.matmul(pda[:], lhs, wda_sb[:, k, :],
                                     start=(k == 0), stop=(k == 1))
                    nc.tensor.matmul(pada[:], lhs, wada_sb[:, k, :],
                                     start=(k == 0), stop=(k == 1))
                xi2 = pe.tile([128, 128], dt.float32, tag="xi2")
                nc.vector.tensor_tensor(xi2[:], pada[:], bada_sb[:], op=Alu.add)
                ln_relu_resid(t_a[:, w, :], pda, ga_sb, bea_sb, bda_sb,
                              xi2[:], 128, pe)
            taT = pec.tile([128, c.RPC], dt.float32r)
            for w in range(c.NW):
                pt = pep.tile([128, 128], dt.float32, tag="ptE")
                nc.tensor.transpose(pt[:], t_a[:, w, :], ident[:])
                nc.vector.tensor_copy(taT[:, w * 128:(w + 1) * 128], pt[:])
            t_1 = pec.tile([128, c.NW, 64], dt.float32)
            for w in range(c.NW):
                pd1 = pep.tile([128, 64], dt.float32, tag="pmA")
                pad1 = pep.tile([128, 64], dt.float32, tag="pmB")
                lhs = taT[:, w * 128:(w + 1) * 128]
                nc.tensor.matmul(pd1[:], lhs, wd1_sb[:], start=True, stop=True)
                nc.tensor.matmul(pad1[:], lhs, wad1_sb[:], start=True,
                                 stop=True)
                xi3 = pe.tile([128, 64], dt.float32, tag="xi3")
                nc.vector.tensor_tensor(xi3[:], pad1[:], bad1_sb[:], op=Alu.add)
                ln_relu_resid(t_1[:, w, :], pd1, g1_sb, be1_sb, bd1_sb,
                              xi3[:], 64, pe)
            t1T = pec.tile([64, c.RPC], dt.float32r)
            for w in range(c.NW):
                pt = pep.tile([128, 128], dt.float32, tag="ptE")
                nc.tensor.transpose(pt[0:64, :], t_1[:, w, :], ident[:])
                nc.vector.tensor_copy(t1T[:, w * 128:(w + 1) * 128],
                                      pt[0:64, :])
            t_2 = pec.tile([128, c.NW, 32], dt.float32)
            for w in range(c.NW):
                pd2 = pep.tile([128, 32], dt.float32, tag="pmA")
                nc.tensor.matmul(pd2[:], t1T[:, w * 128:(w + 1) * 128],
                                 wd2_sb[:], start=True, stop=True)
                ln_relu_resid(t_2[:, w, :], pd2, g2_sb, be2_sb, bd2_sb,
                              None, 32, pe)
            t2T = pec.tile([32, c.RPC], dt.float32r)
            for w in range(c.NW):
                pt = pep.tile([128, 128], dt.float32, tag="ptE")
                nc.tensor.transpose(pt[0:32, :], t_2[:, w, :], ident[:])
                nc.vector.tensor_copy(t2T[:, w * 128:(w + 1) * 128],
                                      pt[0:32, :])
            c_ext = pec.tile([128, c.NW, 4], dt.float32)
            for w in range(c.NW):
                pd3 = pep.tile([128, 4], dt.float32, tag="pmA")
                nc.tensor.matmul(pd3[:], t2T[:, w * 128:(w + 1) * 128],
                                 wd3_sb[:], start=True, stop=True)
                cc = pe.tile([128, 3], dt.float32, tag="cc")
                nc.vector.tensor_tensor(cc[:], pd3[:, 0:3], bd3_sb[:], op=Alu.add)
                nc.vector.tensor_copy(c_ext[:, w, 0:3], cc[:])
                sqc = pe.tile([128, 3], dt.float32, tag="sqc")
                nc.scalar.activation(sqc[:], cc[:], Act.Square,
                                     accum_out=sq_own[:, w:w + 1])
                nc.vector.tensor_copy(c_ext[:, w, 3:4], sq_own[:, w:w + 1])
            cdT = pec.tile([4, c.RPC], dt.float32)
            for w in range(c.NW):
                pt = pep.tile([128, 128], dt.float32, tag="ptE")
                nc.tensor.transpose(pt[0:4, :], c_ext[:, w, :], ident[:])
                nc.vector.tensor_copy(cdT[:, w * 128:(w + 1) * 128],
                                      pt[0:4, :])
            nc.sync.dma_start(cdT_out.ap(), cdT[:])
            nc.sync.dma_start(sq_out.ap(), sq_own[:])

    nc.compile()
    return nc


def build_cdist(c: Cfg):
    nc = bacc.Bacc("TRN2", target_bir_lowering=False, debug=False,
                   num_devices=NCORES)
    rhs_in = nc.dram_tensor("rhs_in", [4, c.N], dt.float32, kind="ExternalInput")
    lhs_in = nc.dram_tensor("lhs_in", [4, c.RPC], dt.float32, kind="ExternalInput")
    sq_in = nc.dram_tensor("sq_in", [128, c.NW], dt.float32, kind="ExternalInput")
    out = nc.dram_tensor("out", [c.RPC, c.N], dt.float32, kind="ExternalOutput")
    with tile.TileContext(nc) as tc:
        with tc.tile_pool(name="pfc", bufs=1) as pfc, \
             tc.tile_pool(name="phf", bufs=3) as pf, \
             tc.tile_pool(name="phf_ps", bufs=2, space="PSUM") as pfp:
            rhs = pfc.tile([4, c.N], dt.float32)
            nc.sync.dma_start(rhs[:], rhs_in.ap())
            lhsT = pfc.tile([4, c.RPC], dt.float32)
            nc.sync.dma_start(lhsT[:], lhs_in.ap())
            sqt = pfc.tile([128, c.NW], dt.float32)
            nc.sync.dma_start(sqt[:], sq_in.ap())
            JC = min(2048, c.N)
            for w in range(c.NW):
                for jc in range(c.N // JC):
                    pcd = pfp.tile([128, JC], dt.float32, tag="pcd")
                    for q in range(JC // 512):
                        nc.tensor.matmul(
                            pcd[:, q * 512:(q + 1) * 512],
                            lhsT[:, w * 128:(w + 1) * 128],
                            rhs[:, jc * JC + q * 512: jc * JC + (q + 1) * 512],
                            start=True, stop=True)
                    d2 = pf.tile([128, JC], dt.float32, tag="d2")
                    nc.vector.tensor_scalar(d2[:], pcd[:], sqt[:, w:w + 1],
                                            0.0, op0=Alu.add, op1=Alu.max)
                    dd = pf.tile([128, JC], dt.float32, tag="dd")
                    nc.scalar.activation(dd[:], d2[:], Act.Sqrt)
                    nc.sync.dma_start(
                        out.ap()[w * 128:(w + 1) * 128, jc * JC:(jc + 1) * JC],
                        dd[:])
    nc.compile()
    return nc


# ----------------------------------------------------------------------------
# host side
# ----------------------------------------------------------------------------
def _wrap_idx(a, EW):
    """[..., EW] int array -> [..., 128, EW//16] int16 dma_gather idx layout
    (j at [16*g + j%16, j//16] for each of the 8 replication groups g)."""
    lead = a.shape[:-1]
    A = a.reshape(*lead, EW // 16, 16).astype(np.int16)     # [..., col, p]
    cols = np.swapaxes(A, -1, -2)                            # [..., p, col]
    out = np.empty((*lead, 128, EW // 16), np.int16)
    for g in range(8):
        out[..., 16 * g:16 * (g + 1), :] = cols
    return out


def prep_inputs(inputs, cfg: Cfg):
    c = cfg
    x = np.ascontiguousarray(np.asarray(inputs["x"], np.float32))
    ei = np.asarray(inputs["edge_index"]).astype(np.int64)
    N = c.N
    src = np.concatenate([ei[0], np.arange(N, dtype=np.int64)])
    dst = np.concatenate([ei[1], np.arange(N, dtype=np.int64)])
    order = np.argsort(dst, kind="stable")
    src, dst = src[order], dst[order]
    nwin_tot = N // 128
    win = (dst // 128).astype(np.int64)
    starts = np.searchsorted(win, np.arange(nwin_tot + 1))

    src_pad = np.zeros((NCORES, c.NW, c.EW), np.int64)
    dst_pad = np.zeros((NCORES, c.NW, c.EW), np.int64)
    drel_pad = np.full((NCORES, c.NW, c.EW), -1.0, np.float32)
    for g in range(nwin_tot):
        core, w = g // c.NW, g % c.NW
        s, e = starts[g], starts[g + 1]
        k = e - s
        assert k <= c.EW, f"window {g} has {k} edges > padded {c.EW}"
        src_pad[core, w, :k] = src[s:e]
        dst_pad[core, w, :k] = dst[s:e]
        drel_pad[core, w, :k] = (dst[s:e] - g * 128).astype(np.float32)

    # weights
    f32 = lambda k: np.asarray(inputs[k], np.float32)
    W1, W2 = f32("W1"), f32("W2")
    a_src1, a_dst1 = f32("a_src1"), f32("a_dst1")
    a_src2, a_dst2 = f32("a_src2"), f32("a_dst2")
    Vs1 = np.stack([W1[:, 0:256] @ a_src1[0], W1[:, 256:512] @ a_src1[1],
                    W1[:, 0:256] @ a_dst1[0], W1[:, 256:512] @ a_dst1[1]],
                   axis=1)                                   # [512, 4]
    Vs2 = np.stack([W2[:, 0:128] @ a_src2[0], W2[:, 128:256] @ a_src2[1],
                    W2[:, 0:128] @ a_dst2[0], W2[:, 128:256] @ a_dst2[1]],
                   axis=1)
    bcast = lambda k, D: np.ascontiguousarray(
        np.broadcast_to(f32(k), (128, D)))
    common = {
        "xT": np.ascontiguousarray(x.T),
        "W1c": np.ascontiguousarray(W1.reshape(4, 128, c.F1)),
        "Vs1": np.ascontiguousarray(Vs1.reshape(4, 128, 4)),
        "W2c": np.ascontiguousarray(W2.reshape(4, 128, c.F2)),
        "Vs2": np.ascontiguousarray(Vs2.reshape(4, 128, 4)),
        "Wal2": np.ascontiguousarray(f32("W_al2").reshape(4, 128, 256)),
        "Wda": np.ascontiguousarray(f32("W_da").reshape(2, 128, 128)),
        "Wada": np.ascontiguousarray(f32("W_ada").reshape(2, 128, 128)),
        "Wd1": f32("W_d1"), "Wad1": f32("W_ad1"),
        "Wd2": f32("W_d2"),
        "Wd3": np.ascontiguousarray(np.pad(f32("W_d3"), ((0, 0), (0, 1)))),
        "Bb1": bcast("b1", c.F1), "Bb2": bcast("b2", c.F2),
        "Bbal2": bcast("b_al2", 256),
        "Bbda": bcast("b_da", 128), "Bbada": bcast("b_ada", 128),
        "Bga": bcast("g_a", 128), "Bbea": bcast("be_a", 128),
        "Bbd1": bcast("b_d1", 64), "Bbad1": bcast("b_ad1", 64),
        "Bg1": bcast("g_1", 64), "Bbe1": bcast("be_1", 64),
        "Bbd2": bcast("b_d2", 32), "Bg2": bcast("g_2", 32),
        "Bbe2": bcast("be_2", 32), "Bbd3": bcast("b_d3", 3),
    }
    in_maps = []
    for core in range(NCORES):
        m = dict(common)
        m["gidx"] = _wrap_idx(src_pad[core], c.EW)
        m["didx"] = _wrap_idx(dst_pad[core], c.EW)
        m["dstrel"] = np.ascontiguousarray(
            drel_pad[core].reshape(c.NW, c.T, 128).transpose(0, 2, 1))
        in_maps.append(m)
    return in_maps


def compute_T(inputs, n):
    ei = np.asarray(inputs["edge_index"]).astype(np.int64)
    dst = np.concatenate([ei[1], np.arange(n, dtype=np.int64)])
    counts = np.bincount(dst // 128, minlength=n // 128)
    return int(np.ceil(counts.max() / 128))


_BUILT = {}


def profile_plan(inputs):
    """Yield (label, nc, in_maps) for each kernel launch, for profiling."""
    n = int(np.asarray(inputs["x"]).shape[0])
    t_win = compute_T(inputs, n)
    cfg = Cfg(n, t_win)
    nc1, nc2 = _BUILT[(n, t_win)]
    in_maps = prep_inputs(inputs, cfg)
    yield "main", nc1, in_maps
    res1 = bass_utils.run_bass_kernel_spmd(nc1, in_maps,
                                           core_ids=list(range(NCORES)))
    cdT = [res1.results[ci]["cdT_out"] for ci in range(NCORES)]
    sqs = [res1.results[ci]["sq_out"] for ci in range(NCORES)]
    rhs = np.concatenate(cdT, axis=1)
    in_maps2 = []
    for ci in range(NCORES):
        lhs = np.empty((4, cfg.RPC), np.float32)
        lhs[0:3] = -2.0 * cdT[ci][0:3]
        lhs[3] = 1.0
        in_maps2.append({"rhs_in": rhs, "lhs_in": lhs, "sq_in": sqs[ci]})
    yield "cdist", nc2, in_maps2


def kernel(**inputs):
    n = int(np.asarray(inputs["x"]).shape[0])
    t_win = compute_T(inputs, n)
    key = (n, t_win)
    if key not in _BUILT:
        _BUILT[key] = (build_kernel(Cfg(n, t_win)), build_cdist(Cfg(n, t_win)))
    nc1, nc2 = _BUILT[key]
    cfg = Cfg(n, t_win)
    in_maps = prep_inputs(inputs, cfg)
    res1 = bass_utils.run_bass_kernel_spmd(nc1, in_maps,
                                           core_ids=list(range(NCORES)))
    # host-side gather of [4, RPC] coord blocks -> [4, N] table + lhsT rows
    cdT = [res1.results[ci]["cdT_out"] for ci in range(NCORES)]
    sqs = [res1.results[ci]["sq_out"] for ci in range(NCORES)]
    rhs = np.concatenate(cdT, axis=1)          # [4, N]: rows cx,cy,cz,sq
    in_maps2 = []
    for ci in range(NCORES):
        lhs = np.empty((4, cfg.RPC), np.float32)
        lhs[0:3] = -2.0 * cdT[ci][0:3]
        lhs[3] = 1.0
        in_maps2.append({"rhs_in": rhs, "lhs_in": lhs, "sq_in": sqs[ci]})
    res2 = bass_utils.run_bass_kernel_spmd(nc2, in_maps2,
                                           core_ids=list(range(NCORES)))
    out = np.concatenate([res2.results[ci]["out"] for ci in range(NCORES)],
                         axis=0)
    np.fill_diagonal(out, 0.0)
    return out



# revision 23
# speedup vs baseline: 2.6051x; 2.6051x over previous
"""Trainium2 Bass kernel for nn_GATNetMultiLayer (2x GAT + dense stack + cdist).

Single merged kernel, dst-node row-partitioned across 8 cores (N/8 rows each).

Per GAT layer the per-edge work is ONE fp16 dma_gather of a packed table row
[feats_h0 | 1 | feats_h1 | 1 | s_src(2) | s_dst(2) | pad]: the ones-columns
give the softmax denominator as extra matmul output columns, the appended
scores remove the separate src-score gather, and per-edge dst scores come
from tiny PE matmuls against host-precomputed one-hot eqT tiles (streamed,
not gathered).  Aggregation = one-hot (sal = onehot(drel)*exp(e)) matmuls.

Tables are exchanged with fp16 AllGathers.  The final cdist is computed
in-kernel after a tiny AllGather of split-fp16 coordinate stripes:
d2 = sq_i + sq_j - 2*c_i.c_j via an 11-row fp16 matmul (exact to ~1e-5),
then one fused Act op sqrt(psum + (sq_i+eps)) writing fp16.
"""
import sys
sys.path.insert(0, "/opt/trn_rl_repo")
import numpy as np

import concourse.bass as bass
import concourse.bacc as bacc
import concourse.mybir as mybir
import concourse.tile as tile
from concourse import bass_utils

dt = mybir.dt
Alu = mybir.AluOpType
Act = mybir.ActivationFunctionType

NCORES = 8
LN_EPS = 1e-5
D2_EPS = 2e-5
F0 = 512
ROW1 = 640   # fp16 elems per L1 table row
ROW2 = 384   # fp16 elems per L2 table row


class Cfg:
    def __init__(self, n, t_win):
        self.N = n
        self.RPC = n // NCORES
        self.NW = self.RPC // 128
        self.T = t_win
        self.EW = t_win * 128


def build_kernel(c: Cfg):
    nc = bacc.Bacc("TRN2", target_bir_lowering=False, debug=False,
                   num_devices=NCORES)

    def inp(name, shape, dtype=dt.float32):
        return nc.dram_tensor(name, shape, dtype, kind="ExternalInput")

    N, NW, T, EW = c.N, c.NW, c.T, c.EW
    NT = N // 128

    xT = inp("xT", [F0, c.RPC], dt.float16)          # own x columns
    gidx = inp("gidx", [NW, 128, EW // 16], dt.int16)
    eqT_in = inp("eqT", [NW, 128, EW], dt.float16)    # one-hot dst tiles
    eq_in = inp("eqS", [NW, 128, EW], dt.float16)     # one-hot edge tiles
    drel = inp("drel", [NW, 128, T])                  # dst-rel or -1
    W1a = inp("W1a", [4, 128, 512], dt.float16)       # W1 (512 cols)
    W1s = inp("W1s", [4, 128, 4], dt.float16)         # Vs1 (4 cols)
    W2a = inp("W2a", [4, 128, 512], dt.float16)       # [W2 | Wal2]
    W2s = inp("W2s", [4, 128, 4], dt.float16)         # Vs2
    Wda2 = inp("Wda2", [2, 128, 256], dt.float16)     # [Wda | Wada]
    Wd1a = inp("Wd1a", [128, 128], dt.float16)        # [Wd1 | Wad1]
    Wd2_ = inp("Wd2_", [64, 32], dt.float16)
    Wd3_ = inp("Wd3_", [32, 4], dt.float16)
    Bb1 = inp("Bb1", [128, 512])
    Bb2 = inp("Bb2", [128, 256])
    Bbal2 = inp("Bbal2", [128, 256])
    Bbda = inp("Bbda", [128, 128])
    Bbada = inp("Bbada", [128, 128])
    Bga = inp("Bga", [128, 128])
    Bbea = inp("Bbea", [128, 128])
    Bbd1 = inp("Bbd1", [128, 64])
    Bbad1 = inp("Bbad1", [128, 64])
    Bg1 = inp("Bg1", [128, 64])
    Bbe1 = inp("Bbe1", [128, 64])
    Bbd2 = inp("Bbd2", [128, 32])
    Bg2 = inp("Bg2", [128, 32])
    Bbe2 = inp("Bbe2", [128, 32])
    Bbd3 = inp("Bbd3", [128, 3])

    out_hw = nc.dram_tensor("out_hw", [c.RPC, N], dt.float16,
                            kind="ExternalOutput")

    ag1_in = nc.dram_tensor("ag1_in", [c.RPC, ROW1], dt.float16)
    h1_full = nc.dram_tensor("h1_full", [N, ROW1], dt.float16,
                             addr_space="Shared")
    ag2_in = nc.dram_tensor("ag2_in", [c.RPC, ROW2], dt.float16)
    h2_full = nc.dram_tensor("h2_full", [N, ROW2], dt.float16,
                             addr_space="Shared")
    ag3_in = nc.dram_tensor("ag3_in", [12, c.RPC], dt.float16)
    ag3_out = nc.dram_tensor("ag3_out", [NCORES * 12, c.RPC], dt.float16,
                             addr_space="Shared")

    rg = [list(range(NCORES))]

    with tile.TileContext(nc) as tc:
      with tc.tile_pool(name="const", bufs=1) as cpool:
        iota_f = cpool.tile([128, 128], dt.float16)
        nc.gpsimd.iota(iota_f[:], pattern=[[1, 128]], base=0,
                       channel_multiplier=0,
                       allow_small_or_imprecise_dtypes=True)
        ident16 = cpool.tile([128, 128], dt.float16)
        ident32 = cpool.tile([128, 128], dt.float32)
        iota_p = cpool.tile([128, 1], dt.float32)
        nc.gpsimd.iota(iota_p[:], pattern=[[0, 1]], base=0,
                       channel_multiplier=1,
                       allow_small_or_imprecise_dtypes=True)
        nc.vector.tensor_scalar(ident16[:], iota_f[:], iota_p[:], None,
                                op0=Alu.is_equal)
        nc.vector.tensor_scalar(ident32[:], iota_f[:], iota_p[:], None,
                                op0=Alu.is_equal)

        h_own = cpool.tile([128, NW, 512], dt.float16)
        xi_own = cpool.tile([128, NW, 256], dt.float16)
        sqe_own = cpool.tile([128, NW], dt.float32)   # sq + D2_EPS
        lhsF = cpool.tile([11, c.RPC], dt.float16)    # cdist lhs stripes
        sdw1 = cpool.tile([128, NW, 2], dt.float16)   # own dst scores L1
        sdw2 = cpool.tile([128, NW, 2], dt.float16)   # own dst scores L2

        # ================= phase A: own h1 rows + scores -> ag1_in ========
        with nc.named_scope("phA"):
         with tc.tile_pool(name="pac", bufs=1) as pac, \
             tc.tile_pool(name="pha", bufs=3) as pa, \
             tc.tile_pool(name="pha_ps", bufs=2, space="PSUM") as pap:
            w1_sb = pac.tile([128, 4, 512], dt.float16)
            nc.sync.dma_start(w1_sb[:], W1a.ap().rearrange("c p f -> p c f"))
            vs1_sb = pac.tile([128, 4, 4], dt.float16)
            nc.sync.dma_start(vs1_sb[:], W1s.ap().rearrange("c p f -> p c f"))
            for w in range(NW):
                xc = pa.tile([128, 4, 128], dt.float16, tag="xc")
                nc.sync.dma_start(
                    xc[:],
                    xT.ap().rearrange("(k p) n -> p k n", p=128)[
                        :, :, w * 128:(w + 1) * 128])
                ph = pap.tile([128, 512], dt.float32, tag="ph")
                psc = pap.tile([128, 4], dt.float32, tag="psc")
                for k in range(4):
                    nc.tensor.matmul(ph[:], xc[:, k, :], w1_sb[:, k, :],
                                     start=(k == 0), stop=(k == 3))
                for k in range(4):
                    nc.tensor.matmul(psc[:], xc[:, k, :], vs1_sb[:, k, :],
                                     start=(k == 0), stop=(k == 3))
                row = pa.tile([128, ROW1], dt.float16, tag="row")
                nc.vector.tensor_copy(row[:, 0:256], ph[:, 0:256])
                nc.vector.tensor_copy(row[:, 257:513], ph[:, 256:512])
                nc.vector.tensor_copy(row[:, 514:518], psc[:])
                nc.vector.tensor_copy(sdw1[:, w, :], psc[:, 2:4])
                nc.vector.memset(row[:, 256:257], 1.0)
                nc.vector.memset(row[:, 513:514], 1.0)
                nc.vector.memset(row[:, 518:640], 0.0)
                nc.sync.dma_start(
                    ag1_in.ap()[w * 128:(w + 1) * 128, :], row[:])
        with nc.named_scope("ag1"):
            nc.gpsimd.collective_compute(
                "AllGather", Alu.bypass, replica_groups=rg,
                ins=[ag1_in.ap().opt()], outs=[h1_full.ap().opt()])

        # ================= GAT window =====================================
        def gat_window(w, tab, ROW, D, score_off, sdw_own, pg, pgp, pgsd, out_cb, pgG=None):
            """Aggregate window w from table `tab` (row width ROW fp16,
            heads of D feats at [0:D],[D+1:2D+1], ones at D,2D+1, scores at
            score_off..+4).  Calls out_cb(pagg0, pagg1) when done."""
            gi = pg.tile([128, EW // 16], dt.int16, tag="gi")
            nc.sync.dma_start(gi[:], gidx.ap()[w, :, :])
            eqT = pg.tile([128, EW], dt.float16, tag="eqT")
            nc.sync.dma_start(eqT[:], eqT_in.ap()[w, :, :])
            eqS = pg.tile([128, T, 128], dt.float16, tag="eqS")
            nc.sync.dma_start(eqS[:], eq_in.ap()[w, :, :])
            G = (pgG or pg).tile([128, T, ROW], dt.float16, tag="G")
            # HW limit: <=1024 idxs per dma_gather call
            for s0 in range(0, T, 8):
                sn = min(8, T - s0)
                nc.gpsimd.dma_gather(G[:, s0:s0 + sn, :], tab.ap(),
                                     gi[:, s0 * 8:(s0 + sn) * 8],
                                     num_idxs=sn * 128, num_idxs_reg=sn * 128,
                                     elem_size=ROW)
            # per-edge dst scores via eqT matmuls -> SD psum [128, T, 2]
            SD = pgsd.tile([128, T, 2], dt.float32, tag="SD")
            for t in range(T):
                nc.tensor.matmul(SD[:, t, :],
                                 eqT[:, t * 128:(t + 1) * 128],
                                 sdw_own[:, w, :],
                                 start=True, stop=True)
            # e = lrelu(sa + sd); ex = exp(e)   [128, T, 2]
            ex = pg.tile([128, T, 2], dt.float32, tag="ex")
            nc.vector.tensor_tensor(
                ex[:], G[:, :, score_off:score_off + 2], SD[:], op=Alu.add)
            nc.vector.scalar_tensor_tensor(ex[:], ex[:], 0.2, ex[:],
                                           op0=Alu.mult, op1=Alu.max)
            nc.scalar.activation(ex[:], ex[:], Act.Exp)
            sal = pg.tile([128, T, 2, 128], dt.float16, tag="sal")
            nc.vector.tensor_tensor(
                sal[:],
                eqS[:].unsqueeze(2).broadcast_to([128, T, 2, 128]),
                ex[:].unsqueeze(3).broadcast_to([128, T, 2, 128]),
                op=Alu.mult)
            pagg0 = pgp.tile([128, D + 1], dt.float32, tag="pagg0")
            pagg1 = pgp.tile([128, D + 1], dt.float32, tag="pagg1")
            pagg = [pagg0, pagg1]
            for t in range(T):
                for h in range(2):
                    nc.tensor.matmul(
                        pagg[h][:], sal[:, t, h, :],
                        G[:, t, h * (D + 1):(h + 1) * (D + 1)],
                        start=(t == 0), stop=(t == T - 1))
            out_cb(pagg0, pagg1)

        # ================= phase B+C: GAT1 + h2/xi/scores2 ================
        with nc.named_scope("phB"):
         with tc.tile_pool(name="pbc", bufs=1) as pbc, \
             tc.tile_pool(name="phb", bufs=2) as pb, \
             tc.tile_pool(name="phb_agg", bufs=2, space="PSUM") as pbp, \
             tc.tile_pool(name="phb_sd", bufs=1, space="PSUM") as pbsd, \
             tc.tile_pool(name="phb_c", bufs=1, space="PSUM") as pbc_ps:
            b1_sb = pbc.tile([128, 512], dt.float32)
            nc.sync.dma_start(b1_sb[:], Bb1.ap())
            bal2_sb = pbc.tile([128, 256], dt.float32)
            nc.sync.dma_start(bal2_sb[:], Bbal2.ap())
            w2_sb = pbc.tile([128, 4, 512], dt.float16)
            nc.sync.dma_start(w2_sb[:], W2a.ap().rearrange("c p f -> p c f"))
            vs2_sb = pbc.tile([128, 4, 4], dt.float16)
            nc.sync.dma_start(vs2_sb[:], W2s.ap().rearrange("c p f -> p c f"))

            def fin1(w):
                def cb(pagg0, pagg1):
                    rcp = pb.tile([128, 2], dt.float32, tag="rcp")
                    nc.vector.reciprocal(rcp[:, 0:1], pagg0[:, 256:257])
                    nc.vector.reciprocal(rcp[:, 1:2], pagg1[:, 256:257])
                    hw_ = h_own[:, w, :]
                    for h, pg_ in ((0, pagg0), (1, pagg1)):
                        nc.vector.scalar_tensor_tensor(
                            hw_[:, h * 256:(h + 1) * 256], pg_[:, 0:256],
                            rcp[:, h:h + 1], b1_sb[:, h * 256:(h + 1) * 256],
                            op0=Alu.mult, op1=Alu.add)
                    nc.vector.tensor_scalar(hw_, hw_, 0.0, None, op0=Alu.max)
                    # ---- phase C for this window ----
                    hT = pb.tile([128, 4, 128], dt.float16, tag="hT")
                    for k in range(4):
                        pt = pbc_ps.tile([128, 128], dt.float16, tag="pt")
                        nc.tensor.transpose(
                            pt[:], hw_[:, k * 128:(k + 1) * 128], ident16[:])
                        nc.vector.tensor_copy(hT[:, k, :], pt[:])
                    pz = pbc_ps.tile([128, 512], dt.float32, tag="pz")
                    ps2 = pbc_ps.tile([128, 64], dt.float32, tag="pt")
                    for k in range(4):
                        nc.tensor.matmul(pz[:], hT[:, k, :], w2_sb[:, k, :],
                                         start=(k == 0), stop=(k == 3))
                    for k in range(4):
                        nc.tensor.matmul(ps2[:, 0:4], hT[:, k, :],
                                         vs2_sb[:, k, :],
                                         start=(k == 0), stop=(k == 3))
                    row = pb.tile([128, ROW2], dt.float16, tag="row2")
                    nc.vector.tensor_copy(row[:, 0:128], pz[:, 0:128])
                    nc.vector.tensor_copy(row[:, 129:257], pz[:, 128:256])
                    nc.vector.tensor_copy(row[:, 258:262], ps2[:, 0:4])
                    nc.vector.tensor_copy(sdw2[:, w, :], ps2[:, 2:4])
                    nc.vector.memset(row[:, 128:129], 1.0)
                    nc.vector.memset(row[:, 257:258], 1.0)
                    nc.vector.memset(row[:, 262:384], 0.0)
                    nc.vector.tensor_tensor(xi_own[:, w, :], pz[:, 256:512],
                                            bal2_sb[:], op=Alu.add)
                    nc.sync.dma_start(
                        ag2_in.ap()[w * 128:(w + 1) * 128, :], row[:])
                return cb

            for w in range(NW):
                gat_window(w, h1_full, ROW1, 256, 514, sdw1, pb, pbp, pbsd, fin1(w))
        with nc.named_scope("ag2"):
            nc.gpsimd.collective_compute(
                "AllGather", Alu.bypass, replica_groups=rg,
                ins=[ag2_in.ap().opt()], outs=[h2_full.ap().opt()])

        # ================= phase D+E: GAT2 + dense stack + coords =========
        with nc.named_scope("phD"):
         with tc.tile_pool(name="pdc", bufs=1) as pdc, \
             tc.tile_pool(name="phd", bufs=2) as pd, \
             tc.tile_pool(name="phdG", bufs=3) as pdG, \
             tc.tile_pool(name="phd_agg", bufs=2, space="PSUM") as pdp, \
             tc.tile_pool(name="phd_sd", bufs=1, space="PSUM") as pdsd, \
             tc.tile_pool(name="phd_e", bufs=1, space="PSUM") as pde:
            b2_sb = pdc.tile([128, 256], dt.float32)
            nc.sync.dma_start(b2_sb[:], Bb2.ap())
            wda_sb = pdc.tile([128, 2, 256], dt.float16)
            nc.sync.dma_start(wda_sb[:], Wda2.ap().rearrange("c p f -> p c f"))
            wd1_sb = pdc.tile([128, 128], dt.float16)
            nc.sync.dma_start(wd1_sb[:], Wd1a.ap())
            wd2_sb = pdc.tile([64, 32], dt.float16)
            nc.sync.dma_start(wd2_sb[:], Wd2_.ap())
            wd3_sb = pdc.tile([32, 4], dt.float16)
            nc.sync.dma_start(wd3_sb[:], Wd3_.ap())
            bda_sb = pdc.tile([128, 128], dt.float32)
            nc.sync.dma_start(bda_sb[:], Bbda.ap())
            bada_sb = pdc.tile([128, 128], dt.float32)
            nc.sync.dma_start(bada_sb[:], Bbada.ap())
            ga_sb = pdc.tile([128, 128], dt.float32)
            nc.sync.dma_start(ga_sb[:], Bga.ap())
            bea_sb = pdc.tile([128, 128], dt.float32)
            nc.sync.dma_start(bea_sb[:], Bbea.ap())
            bd1_sb = pdc.tile([128, 64], dt.float32)
            nc.sync.dma_start(bd1_sb[:], Bbd1.ap())
            bad1_sb = pdc.tile([128, 64], dt.float32)
            nc.sync.dma_start(bad1_sb[:], Bbad1.ap())
            g1_sb = pdc.tile([128, 64], dt.float32)
            nc.sync.dma_start(g1_sb[:], Bg1.ap())
            be1_sb = pdc.tile([128, 64], dt.float32)
            nc.sync.dma_start(be1_sb[:], Bbe1.ap())
            bd2_sb = pdc.tile([128, 32], dt.float32)
            nc.sync.dma_start(bd2_sb[:], Bbd2.ap())
            g2_sb = pdc.tile([128, 32], dt.float32)
            nc.sync.dma_start(g2_sb[:], Bg2.ap())
            be2_sb = pdc.tile([128, 32], dt.float32)
            nc.sync.dma_start(be2_sb[:], Bbe2.ap())
            bd3_sb = pdc.tile([128, 3], dt.float32)
            nc.sync.dma_start(bd3_sb[:], Bbd3.ap())

            def ln_relu(x, g_sb, be_sb, D, out_ap, xi_ap, pe):
                """out = relu(LN(x)*g+be) (+ xi).  x: [128, D] fp32 SBUF."""
                s1 = pe.tile([128, 1], dt.float32, tag=f"ls{D}")
                sq = pe.tile([128, D], dt.float32, tag=f"lq{D}")
                ss = pe.tile([128, 1], dt.float32, tag=f"lss{D}")
                nc.scalar.activation(sq[:], x[:], Act.Square, accum_out=ss[:])
                nc.vector.reduce_sum(s1[:], x[:], axis=mybir.AxisListType.X)
                mu = pe.tile([128, 1], dt.float32, tag=f"lm{D}")
                nc.vector.tensor_scalar(mu[:], s1[:], 1.0 / D, None,
                                        op0=Alu.mult)
                v1 = pe.tile([128, 1], dt.float32, tag=f"lv{D}")
                nc.vector.tensor_scalar(v1[:], ss[:], 1.0 / D, None,
                                        op0=Alu.mult)
                mu2 = pe.tile([128, 1], dt.float32, tag=f"lm2{D}")
                nc.vector.tensor_scalar(mu2[:], mu[:], mu[:], None,
                                        op0=Alu.mult)
                var = pe.tile([128, 1], dt.float32, tag=f"lvr{D}")
                nc.vector.scalar_tensor_tensor(var[:], v1[:], LN_EPS, mu2[:],
                                               op0=Alu.add, op1=Alu.subtract)
                std = pe.tile([128, 1], dt.float32, tag=f"lsd{D}")
                nc.scalar.activation(std[:], var[:], Act.Sqrt)
                rstd = pe.tile([128, 1], dt.float32, tag=f"lrs{D}")
                nc.vector.reciprocal(rstd[:], std[:])
                nrm = pe.tile([128, D], dt.float32, tag=f"lnr{D}")
                nc.vector.tensor_scalar(nrm[:], x[:], mu[:], rstd[:],
                                        op0=Alu.subtract, op1=Alu.mult)
                nc.vector.tensor_tensor(nrm[:], nrm[:], g_sb[:, 0:D],
                                        op=Alu.mult)
                nc.vector.tensor_tensor(nrm[:], nrm[:], be_sb[:, 0:D],
                                        op=Alu.add)
                if xi_ap is not None:
                    nc.vector.scalar_tensor_tensor(out_ap, nrm[:], 0.0, xi_ap,
                                                   op0=Alu.max, op1=Alu.add)
                else:
                    nc.vector.tensor_scalar(out_ap, nrm[:], 0.0, None,
                                            op0=Alu.max)

            def fin2(w):
                def cb(pagg0, pagg1):
                    rcp = pd.tile([128, 2], dt.float32, tag="rcp")
                    nc.vector.reciprocal(rcp[:, 0:1], pagg0[:, 128:129])
                    nc.vector.reciprocal(rcp[:, 1:2], pagg1[:, 128:129])
                    h2f = pd.tile([128, 256], dt.float16, tag="h2f")
                    for h, pg_ in ((0, pagg0), (1, pagg1)):
                        nc.vector.scalar_tensor_tensor(
                            h2f[:, h * 128:(h + 1) * 128], pg_[:, 0:128],
                            rcp[:, h:h + 1], b2_sb[:, h * 128:(h + 1) * 128],
                            op0=Alu.mult, op1=Alu.add)
                    nc.vector.scalar_tensor_tensor(
                        h2f[:], h2f[:], 0.0, xi_own[:, w, :],
                        op0=Alu.max, op1=Alu.add)
                    # dense stack
                    h2T = pd.tile([128, 2, 128], dt.float16, tag="h2T")
                    for k in range(2):
                        pt = pde.tile([128, 128], dt.float16, tag="ptE16")
                        nc.tensor.transpose(
                            pt[:], h2f[:, k * 128:(k + 1) * 128], ident16[:])
                        nc.vector.tensor_copy(h2T[:, k, :], pt[:])
                    pda = pde.tile([128, 256], dt.float32, tag="pmA")
                    for k in range(2):
                        nc.tensor.matmul(pda[:], h2T[:, k, :], wda_sb[:, k, :],
                                         start=(k == 0), stop=(k == 1))
                    xa = pd.tile([128, 128], dt.float32, tag="xa")
                    nc.vector.tensor_tensor(xa[:], pda[:, 0:128], bda_sb[:],
                                            op=Alu.add)
                    xi2 = pd.tile([128, 128], dt.float32, tag="xi2")
                    nc.vector.tensor_tensor(xi2[:], pda[:, 128:256],
                                            bada_sb[:], op=Alu.add)
                    t_a = pd.tile([128, 128], dt.float16, tag="t_a")
                    ln_relu(xa, ga_sb, bea_sb, 128, t_a[:], xi2[:], pd)
                    ptA = pde.tile([128, 128], dt.float16, tag="ptE16")
                    nc.tensor.transpose(ptA[:], t_a[:], ident16[:])
                    taT = pd.tile([128, 128], dt.float16, tag="taT")
                    nc.vector.tensor_copy(taT[:], ptA[:])
                    pd1 = pde.tile([128, 128], dt.float32, tag="pmA")
                    nc.tensor.matmul(pd1[:], taT[:], wd1_sb[:],
                                     start=True, stop=True)
                    x1 = pd.tile([128, 64], dt.float32, tag="x1")
                    nc.vector.tensor_tensor(x1[:], pd1[:, 0:64], bd1_sb[:],
                                            op=Alu.add)
                    xi3 = pd.tile([128, 64], dt.float32, tag="xi3")
                    nc.vector.tensor_tensor(xi3[:], pd1[:, 64:128],
                                            bad1_sb[:], op=Alu.add)
                    t_1 = pd.tile([128, 64], dt.float16, tag="t_1")
                    ln_relu(x1, g1_sb, be1_sb, 64, t_1[:], xi3[:], pd)
                    pt1 = pde.tile([128, 128], dt.float16, tag="ptE16")
                    nc.tensor.transpose(pt1[0:64, :], t_1[:], ident16[:])
                    t1T = pd.tile([64, 128], dt.float16, tag="t1T")
                    nc.vector.tensor_copy(t1T[:], pt1[0:64, :])
                    pd2 = pde.tile([128, 32], dt.float32, tag="pmA")
                    nc.tensor.matmul(pd2[:], t1T[:], wd2_sb[:],
                                     start=True, stop=True)
                    x2 = pd.tile([128, 32], dt.float32, tag="x2")
                    nc.vector.tensor_tensor(x2[:], pd2[:], bd2_sb[:],
                                            op=Alu.add)
                    t_2 = pd.tile([128, 32], dt.float16, tag="t_2")
                    ln_relu(x2, g2_sb, be2_sb, 32, t_2[:], None, pd)
                    pt2 = pde.tile([128, 128], dt.float16, tag="ptE16")
                    nc.tensor.transpose(pt2[0:32, :], t_2[:], ident16[:])
                    t2T = pd.tile([32, 128], dt.float16, tag="t2T")
                    nc.vector.tensor_copy(t2T[:], pt2[0:32, :])
                    pd3 = pde.tile([128, 4], dt.float32, tag="pmA")
                    nc.tensor.matmul(pd3[:], t2T[:], wd3_sb[:],
                                     start=True, stop=True)
                    cc = pd.tile([128, 3], dt.float32, tag="cc")
                    nc.vector.tensor_tensor(cc[:], pd3[:, 0:3], bd3_sb[:],
                                            op=Alu.add)
                    sqc = pd.tile([128, 3], dt.float32, tag="sqc")
                    sqv = pd.tile([128, 1], dt.float32, tag="sqv")
                    nc.scalar.activation(sqc[:], cc[:], Act.Square,
                                         accum_out=sqv[:])
                    nc.vector.tensor_scalar(sqe_own[:, w:w + 1], sqv[:],
                                            D2_EPS, None, op0=Alu.add)
                    # split-fp16 stripes.  rhs k-rows (a16 cols, then
                    # transposed): [a(3) a(3) b(3) sqa sqb pad]; lhs k-rows
                    # (m2): [-2a(3) -2b(3) -2a(3) 1 1 pad] so out =
                    # sum_k lhs*rhs = -2(a.a + b.a + a.b) + sqa + sqb.
                    a16 = pd.tile([128, 12], dt.float16, tag="a16")
                    nc.vector.tensor_copy(a16[:, 0:3], cc[:])
                    nc.vector.tensor_copy(a16[:, 3:6], cc[:])
                    nc.vector.tensor_copy(a16[:, 9:10], sqv[:])
                    b32 = pd.tile([128, 4], dt.float32, tag="b32")
                    nc.vector.tensor_tensor(b32[:, 0:3], cc[:], a16[:, 0:3],
                                            op=Alu.subtract)
                    nc.vector.tensor_tensor(b32[:, 3:4], sqv[:], a16[:, 9:10],
                                            op=Alu.subtract)
                    nc.vector.tensor_copy(a16[:, 6:9], b32[:, 0:3])
                    nc.vector.tensor_copy(a16[:, 10:11], b32[:, 3:4])
                    nc.vector.memset(a16[:, 11:12], 0.0)
                    m2 = pd.tile([128, 12], dt.float16, tag="m2")
                    nc.vector.tensor_scalar(m2[:, 0:3], a16[:, 0:3], -2.0,
                                            None, op0=Alu.mult)
                    nc.vector.tensor_scalar(m2[:, 3:6], a16[:, 6:9], -2.0,
                                            None, op0=Alu.mult)
                    nc.vector.tensor_copy(m2[:, 6:9], m2[:, 0:3])
                    nc.vector.memset(m2[:, 9:11], 1.0)
                    nc.vector.memset(m2[:, 11:12], 0.0)
                    ptS = pde.tile([128, 128], dt.float16, tag="ptE16")
                    nc.tensor.transpose(ptS[0:12, :], a16[:], ident16[:])
                    rT = pd.tile([12, 128], dt.float16, tag="rT")
                    nc.vector.tensor_copy(rT[:], ptS[0:12, :])
                    nc.sync.dma_start(
                        ag3_in.ap()[:, w * 128:(w + 1) * 128], rT[:])
                    ptL = pde.tile([128, 128], dt.float16, tag="ptE16")
                    nc.tensor.transpose(ptL[0:12, :], m2[:], ident16[:])
                    nc.vector.tensor_copy(lhsF[0:11, w * 128:(w + 1) * 128],
                                          ptL[0:11, :])
                return cb

            for w in range(NW):
                gat_window(w, h2_full, ROW2, 128, 258, sdw2, pd, pdp, pdsd, fin2(w), pgG=pdG)
        with nc.named_scope("ag3"):
            nc.gpsimd.collective_compute(
                "AllGather", Alu.bypass, replica_groups=rg,
                ins=[ag3_in.ap().opt()], outs=[ag3_out.ap().opt()])

        # ================= phase F: cdist row block =======================
        with nc.named_scope("phF"):
         with tc.tile_pool(name="pfc", bufs=1) as pfc, \
             tc.tile_pool(name="phf", bufs=3) as pf, \
             tc.tile_pool(name="phf_ps", bufs=2, space="PSUM") as pfp:
            # rhs [11, N]: k-rows [a(3) a(3) b(3) sqa sqb] per col block
            rhs = pfc.tile([11, N], dt.float16)
            for cid in range(NCORES):
                nc.sync.dma_start(
                    rhs[:, cid * c.RPC:(cid + 1) * c.RPC],
                    ag3_out.ap()[cid * 12:cid * 12 + 11, :])
            JC = 2048
            for w in range(NW):
                for jc in range(N // JC):
                    pcd = pfp.tile([128, JC], dt.float32, tag="pcd")
                    for q in range(JC // 512):
                        nc.tensor.matmul(
                            pcd[:, q * 512:(q + 1) * 512],
                            lhsF[:, w * 128:(w + 1) * 128],
                            rhs[:, jc * JC + q * 512:jc * JC + (q + 1) * 512],
                            start=True, stop=True)
                    dd = pf.tile([128, JC], dt.float16, tag="dd")
                    nc.scalar.activation(dd[:], pcd[:], Act.Sqrt,
                                         bias=sqe_own[:, w:w + 1])
                    nc.sync.dma_start(
                        out_hw.ap()[w * 128:(w + 1) * 128,
                                    jc * JC:(jc + 1) * JC], dd[:])

    nc.compile()
    return nc


# ----------------------------------------------------------------------------
# host side
# ----------------------------------------------------------------------------
def _wrap_idx(a, EW):
    """[..., EW] int array -> [..., 128, EW//16] int16 dma_gather idx layout."""
    lead = a.shape[:-1]
    A = a.reshape(*lead, EW // 16, 16).astype(np.int16)
    cols = np.swapaxes(A, -1, -2)
    out = np.empty((*lead, 128, EW // 16), np.int16)
    for g in range(8):
        out[..., 16 * g:16 * (g + 1), :] = cols
    return out


def compute_T(inputs, n):
    ei = np.asarray(inputs["edge_index"]).astype(np.int64)
    dst = np.concatenate([ei[1], np.arange(n, dtype=np.int64)])
    counts = np.bincount(dst // 128, minlength=n // 128)
    return int(np.ceil(counts.max() / 128))


def prep_inputs(inputs, cfg: Cfg):
    c = cfg
    N, NW, T, EW = c.N, c.NW, c.T, c.EW
    x = np.asarray(inputs["x"], np.float32)
    ei = np.asarray(inputs["edge_index"]).astype(np.int64)
    src = np.concatenate([ei[0], np.arange(N, dtype=np.int64)])
    dst = np.concatenate([ei[1], np.arange(N, dtype=np.int64)])
    order = np.argsort(dst, kind="stable")
    src, dst = src[order], dst[order]
    nwin_tot = N // 128
    win = (dst // 128).astype(np.int64)
    starts = np.searchsorted(win, np.arange(nwin_tot + 1))

    src_pad = np.zeros((NCORES, NW, EW), np.int64)
    drel_pad = np.full((NCORES, NW, EW), -1.0, np.float32)
    for g in range(nwin_tot):
        core, w = g // NW, g % NW
        s, e = starts[g], starts[g + 1]
        k = e - s
        assert k <= EW, f"window {g} has {k} edges > padded {EW}"
        src_pad[core, w, :k] = src[s:e]
        drel_pad[core, w, :k] = (dst[s:e] - g * 128).astype(np.float32)

    f32 = lambda k: np.asarray(inputs[k], np.float32)
    f16 = lambda a: np.ascontiguousarray(a.astype(np.float16))
    W1, W2 = f32("W1"), f32("W2")
    a_src1, a_dst1 = f32("a_src1"), f32("a_dst1")
    a_src2, a_dst2 = f32("a_src2"), f32("a_dst2")
    Vs1 = np.stack([W1[:, 0:256] @ a_src1[0], W1[:, 256:512] @ a_src1[1],
                    W1[:, 0:256] @ a_dst1[0], W1[:, 256:512] @ a_dst1[1]],
                   axis=1)                                   # [512, 4]
    Vs2 = np.stack([W2[:, 0:128] @ a_src2[0], W2[:, 128:256] @ a_src2[1],
                    W2[:, 0:128] @ a_dst2[0], W2[:, 128:256] @ a_dst2[1]],
                   axis=1)                                   # [512, 4]
    W2al = np.concatenate([W2, f32("W_al2")], axis=1)        # [512, 512]
    Wda2 = np.concatenate([f32("W_da"), f32("W_ada")], axis=1)  # [256, 256]
    Wd1a = np.concatenate([f32("W_d1"), f32("W_ad1")], axis=1)  # [128, 128]
    bcast = lambda k, D: np.ascontiguousarray(
        np.broadcast_to(f32(k), (128, D)))
    common = {
        "W1a": f16(W1.reshape(4, 128, 512)),
        "W1s": f16(Vs1.reshape(4, 128, 4)),
        "W2a": f16(W2al.reshape(4, 128, 512)),
        "W2s": f16(Vs2.reshape(4, 128, 4)),
        "Wda2": f16(Wda2.reshape(2, 128, 256)),
        "Wd1a": f16(Wd1a),
        "Wd2_": f16(f32("W_d2")),
        "Wd3_": f16(np.pad(f32("W_d3"), ((0, 0), (0, 1)))),
        "Bb1": bcast("b1", 512), "Bb2": bcast("b2", 256),
        "Bbal2": bcast("b_al2", 256),
        "Bbda": bcast("b_da", 128), "Bbada": bcast("b_ada", 128),
        "Bga": bcast("g_a", 128), "Bbea": bcast("be_a", 128),
        "Bbd1": bcast("b_d1", 64), "Bbad1": bcast("b_ad1", 64),
        "Bg1": bcast("g_1", 64), "Bbe1": bcast("be_1", 64),
        "Bbd2": bcast("b_d2", 32), "Bg2": bcast("g_2", 32),
        "Bbe2": bcast("be_2", 32), "Bbd3": bcast("b_d3", 3),
    }
    xT16 = np.ascontiguousarray(x.T.astype(np.float16))      # [512, N]
    in_maps = []
    for core in range(NCORES):
        m = dict(common)
        m["xT"] = np.ascontiguousarray(
            xT16[:, core * c.RPC:(core + 1) * c.RPC])
        m["gidx"] = _wrap_idx(src_pad[core], EW)
        dr = drel_pad[core].reshape(NW, T, 128).transpose(0, 2, 1)
        m["drel"] = np.ascontiguousarray(dr)                 # [NW, 128, T]
        # eqT[w, j, t*128+p] = (j == drel at edge j=t*128+p)
        eqT = np.zeros((NW, 128, EW), np.float16)
        eqS = np.zeros((NW, 128, EW), np.float16)
        flat = drel_pad[core].astype(np.int64)               # [NW, EW] j-order
        cols = np.arange(EW)
        for w in range(NW):
            valid = flat[w] >= 0
            eqT[w, flat[w][valid], cols[valid]] = 1.0
            j = cols[valid]
            eqS[w, j % 128, (j // 128) * 128 + flat[w][valid]] = 1.0
        m["eqT"] = eqT
        m["eqS"] = eqS
        in_maps.append(m)
    return in_maps


_BUILT = {}


def _get(inputs):
    n = int(np.asarray(inputs["x"]).shape[0])
    t_win = compute_T(inputs, n)
    key = (n, t_win)
    if key not in _BUILT:
        _BUILT[key] = build_kernel(Cfg(n, t_win))
    return _BUILT[key], Cfg(n, t_win)


def profile_plan(inputs):
    nc, cfg = _get(inputs)
    yield "main", nc, prep_inputs(inputs, cfg)


def kernel(**inputs):
    nc, cfg = _get(inputs)
    in_maps = prep_inputs(inputs, cfg)
    res = bass_utils.run_bass_kernel_spmd(nc, in_maps,
                                          core_ids=list(range(NCORES)))
    out = np.concatenate(
        [res.results[ci]["out_hw"] for ci in range(NCORES)],
        axis=0).astype(np.float32)
    np.fill_diagonal(out, 0.0)
    return out
